# revision 1
# baseline (speedup 1.0000x reference)
"""MoE routing kernel v2 (Plan Z): TensorEngine planar pipeline.

out = y1 + r*yd,  y1 = x@W1.T+b1, yd = x@Wd.T+bd, Wd=W2-W1, r=route.

Per-core layout: tokens t = p*2048 + i*R + l  (p partition, i tile, l low).
 xr [128, R, 11] f32: cols 0-9 = x features, col 10 = r  (built on-chip).
 Chunks of 11 lows (121 free cols); PE-transpose (fp32 fast mode) to planar
 psum rows 11g+f, plus a ones-row at 121.  Three bf16 matmuls vs the same
 moving x_pl: S_A -> y1 (+b1 via ones-row), S_B -> yd (+bd), S_R -> r_pl
 (replicates each group's r-row across its 10 output rows).  DVE 2-op
 select in planar, fp32 PE back-transpose, DVE drains token-major psum
 into the f32 out tile.
"""

import numpy as np

import concourse.bacc as bacc
import concourse.mybir as mybir
from concourse.tile import TileContext
from concourse.masks import make_identity
from concourse.bass_utils import run_bass_kernel_spmd

F32 = mybir.dt.float32
BF16 = mybir.dt.bfloat16
I32 = mybir.dt.int32

N_CORES = 8
P = 128
D = 10
DF = 11            # features incl. r column
KK = 121           # planar rows: 11 groups * 11
MM = 110           # out rows: 11 groups * 10


def pack_wt(W1, b1, W2, b2):
    """[128, 662] f32: cols 0:110 S_A, 110:220 S_B, 220:330 S_R,
    330 = bd col, 331 = b1 col, 332:442 S_A ragged(6 groups),
    442:552 S_B ragged, 552:662 S_R ragged."""
    Wd = W2.astype(np.float64) - W1.astype(np.float64)
    bd = b2.astype(np.float64) - b1.astype(np.float64)
    out = np.zeros((P, 662), np.float32)

    def fill(dst, Wm, rrow, groups=11):
        for g in range(groups):
            for u in range(D):
                if rrow:
                    dst[DF * g + D, D * g + u] = 1.0
                else:
                    for k in range(D):
                        dst[DF * g + k, D * g + u] = Wm[u, k]

    for base, gn in ((0, 11), (332, 6)):
        SA = np.zeros((P, MM), np.float64); fill(SA, W1, False, gn)
        SB = np.zeros((P, MM), np.float64); fill(SB, Wd, False, gn)
        SR = np.zeros((P, MM), np.float64); fill(SR, None, True, gn)
        out[:, base:base + 110] = SA
        out[:, base + 110:base + 220] = SB
        out[:, base + 220:base + 330] = SR
    out[:MM, 330] = np.tile(bd, 11)
    out[:MM, 331] = np.tile(b1, 11)
    return out


def build_moe_pe(tc_tokens, r_tile=512, reps=1, drains=("vector", "scalar",
                                                        "scalar"),
                 pbufs=(3, 2, 1), ileave="vector", stage=3, dma_ileave=False):
    """drains = engines for (x_pl copy, r_pl copy, final out copy)."""
    R = r_tile
    assert tc_tokens % (P * R) == 0
    nt = tc_tokens // (P * R)
    FR = R * D           # f32 free cols per partition of x / out tiles
    FRX = R * DF         # free cols of interleaved xr
    NCH = FRX // (11 * DF)          # full 11-low chunks per tile
    rag_lows = R - NCH * 11         # leftover lows
    
    nc = bacc.Bacc("TRN2", target_bir_lowering=False, debug=False,
                   num_devices=N_CORES)
    x_ext = nc.dram_tensor("x", [tc_tokens, D], F32, kind="ExternalInput")
    r_ext = nc.dram_tensor("route", [tc_tokens], I32, kind="ExternalInput")
    w_ext = nc.dram_tensor("wt", [P, 662], F32, kind="ExternalInput")
    o_ext = nc.dram_tensor("out", [tc_tokens, D], F32, kind="ExternalOutput")

    xv = x_ext.rearrange("(n p r) d -> n p (r d)", p=P, r=R)
    xvs = x_ext.rearrange("(n p r) d -> n p r d", p=P, r=R)
    rv = r_ext.rearrange("(n p r) -> n p r", p=P, r=R)
    ov = o_ext.rearrange("(n p r) d -> n p (r d)", p=P, r=R)

    # chunk groups of up to 4 full chunks, then one ragged group
    groups = []
    c = 0
    while c < NCH:
        ng = min(4, NCH - c)
        groups.append((c, ng))
        c += ng

    with TileContext(nc) as tc:
        with tc.tile_pool(name="const", bufs=1) as cpool, \
             tc.tile_pool(name="sbuf", bufs=3) as pool, \
             tc.tile_pool(name="sbo", bufs=2) as pool2, \
             tc.psum_pool(name="ppx", bufs=pbufs[0]) as ppx, \
             tc.psum_pool(name="ppm", bufs=1) as ppm:
            def drain_copy(which, out, in_):
                if which == "scalar":
                    nc.scalar.copy(out=out, in_=in_)
                else:
                    nc.vector.tensor_copy(out=out, in_=in_)
            idf = cpool.tile([P, P], F32)
            make_identity(nc, idf[:])

            wt = cpool.tile([P, 662], F32)
            nc.sync.dma_start(out=wt[:], in_=w_ext[:])
            SA = cpool.tile([KK, MM], BF16)
            nc.vector.tensor_copy(out=SA[:], in_=wt[:KK, 0:110])
            SB = cpool.tile([KK, MM], BF16)
            nc.vector.tensor_copy(out=SB[:], in_=wt[:KK, 110:220])
            SR = cpool.tile([KK, MM], BF16)
            nc.vector.tensor_copy(out=SR[:], in_=wt[:KK, 220:330])
            SAr = cpool.tile([KK, MM], BF16)
            nc.vector.tensor_copy(out=SAr[:], in_=wt[:KK, 332:442])
            SBr = cpool.tile([KK, MM], BF16)
            nc.vector.tensor_copy(out=SBr[:], in_=wt[:KK, 442:552])
            SRr = cpool.tile([KK, MM], BF16)
            nc.vector.tensor_copy(out=SRr[:], in_=wt[:KK, 552:662])

            for _ in range(reps):
                stageB = []   # deferred per-group closures (one iter later)
                stageC = []   # two iters later

                def run_deferred(lists):
                    for q in lists:
                        while q:
                            q.pop(0)()

                for i in range(nt):
                    xt = pool.tile([P, FR], F32, tag="xt", bufs=2)
                    nc.sync.dma_start(out=xt[:], in_=xv[i])
                    rt = pool.tile([P, R], I32, tag="rt")
                    nc.sync.dma_start(out=rt[:], in_=rv[i])
                    xr = pool.tile([P, R, DF], F32, tag="xr", bufs=2)
                    xtv = xt[:].rearrange("p (r d) -> p r d", d=D)
                    if ileave == "scalar":
                        nc.scalar.copy(out=xr[:, :, 0:D], in_=xtv[:])
                    elif ileave == "split":
                        h = R // 2
                        nc.scalar.copy(out=xr[:, :h, 0:D], in_=xtv[:, :h])
                        nc.vector.tensor_copy(out=xr[:, h:, 0:D],
                                              in_=xtv[:, h:])
                    else:
                        nc.vector.tensor_copy(out=xr[:, :, 0:D], in_=xtv[:])
                    nc.vector.tensor_copy(out=xr[:, :, D], in_=rt[:])
                    xrf = xr[:].rearrange("p r d -> p (r d)")

                    out_tile = pool2.tile([P, FR], F32, tag="ot")
                    glist = groups + ([(NCH, 0)] if rag_lows else [])
                    last_gi = len(glist) - 1

                    for gi, (c0, ng) in enumerate(glist):
                        rag = ng == 0
                        N = 128 * (ng if not rag else 1)
                        nc4 = ng if not rag else 1
                        # ---- stage A (group g): transposes, drain0, mms
                        ps_x = ppx.tile([P, 512], F32, tag="pst")
                        for k in range(nc4):
                            ch = c0 + k
                            fl = 11 * DF if not rag else rag_lows * DF
                            nc.tensor.transpose(
                                ps_x[0:fl, 128 * k:128 * k + 128],
                                xrf[:, 121 * ch:121 * ch + fl], idf[:])
                        x_pl = pool.tile([KK, 512], BF16, tag="xpl")
                        d0 = drains[0] if drains[0] != "mix" else \
                            ("scalar" if gi % 2 == 0 else "vector")
                        drain_copy(d0, x_pl[:, :N], ps_x[:KK, :N])
                        ps_ab = ppm.tile([MM, 1024], F32, tag="psab",
                                         bufs=pbufs[1])
                        ps_r = ppm.tile([MM, 512], F32, tag="psr",
                                        bufs=pbufs[2])
                        Sa, Sb, Sr = (SA, SB, SR) if not rag else \
                            (SAr, SBr, SRr)
                        nc.tensor.matmul(ps_r[:, 0:N], Sr[:], x_pl[:, :N])
                        nc.tensor.matmul(ps_ab[:, 0:N], Sa[:], x_pl[:, :N])
                        nc.tensor.matmul(ps_ab[:, 512:512 + N], Sb[:],
                                         x_pl[:, :N])

                        # ---- deferred stage B for this group
                        def mk_stageB(ps_ab=ps_ab, ps_r=ps_r, N=N, nc4=nc4,
                                      c0=c0, rag=rag, out_tile=out_tile, gi=gi):
                            def f():
                                ALU = mybir.AluOpType
                                r_pl = pool.tile([MM, 512], F32, tag="rpl")
                                d1 = drains[1] if drains[1] != "mix" else \
                                    ("vector" if gi % 2 == 0 else "scalar")
                                drain_copy(d1, r_pl[:, :N], ps_r[:, :N])
                                m_sb = pool.tile([MM, 512], F32, tag="msb")
                                nc.vector.scalar_tensor_tensor(
                                    out=m_sb[:, :N],
                                    in0=ps_ab[:, 512:512 + N],
                                    scalar=wt[0:MM, 330:331],
                                    in1=r_pl[:, :N],
                                    op0=ALU.add, op1=ALU.mult)
                                out_pl = pool.tile([P, 512], F32, tag="opl")
                                nc.vector.scalar_tensor_tensor(
                                    out=out_pl[:MM, :N], in0=ps_ab[:, 0:N],
                                    scalar=wt[0:MM, 331:332],
                                    in1=m_sb[:, :N],
                                    op0=ALU.add, op1=ALU.add)
                                ps_o = ppx.tile([P, 512], F32, tag="pst")
                                for k in range(nc4):
                                    nc.tensor.transpose(
                                        ps_o[:, 128 * k:128 * k + 128],
                                        out_pl[:, 128 * k:128 * k + 128],
                                        idf[:])
                                # deferred stage C: final drain
                                def g():
                                    nlow = 11 if not rag else rag_lows
                                    KB = nlow * D
                                    ob = 110 * c0
                                    wid = 110 * nc4 if not rag else KB
                                    pov = ps_o[:].rearrange(
                                        "p (k c) -> p k c", c=128)
                                    d2 = drains[2] if drains[2] != "mix" \
                                        else ("scalar" if gi % 2 == 0
                                              else "vector")
                                    drain_copy(d2,
                                               out_tile[:, ob:ob + wid],
                                               pov[:, :nc4, :KB])
                                stageC.append(g)
                            return f
                        stageB.append(mk_stageB())

                        # run one deferred closure from each older stage
                        if stageC:
                            stageC.pop(0)()
                        if len(stageB) > 1:
                            stageB.pop(0)()

                    def mk_dma(i=i, out_tile=out_tile):
                        def f():
                            nc.sync.dma_start(out=ov[i], in_=out_tile[:])
                        return f
                    if i == nt - 1:
                        run_deferred([stageB, stageC])
                        mk_dma()()
                    else:
                        # emitted via B-queue so it lands after this tile's
                        # last stage-C drain in the C-queue
                        stageB.append(
                            lambda f=mk_dma(): stageC.append(f))
    nc.compile()
    return nc


def run_sharded(nc, x, route, tc_tokens, wt):
    in_maps = []
    for c in range(N_CORES):
        sl = slice(c * tc_tokens, (c + 1) * tc_tokens)
        in_maps.append({"x": np.ascontiguousarray(x[sl]),
                        "route": np.ascontiguousarray(route[sl]),
                        "wt": wt})
    res = run_bass_kernel_spmd(nc, in_maps, core_ids=list(range(N_CORES)))
    return np.concatenate([res.results[c]["out"] for c in range(N_CORES)],
                          axis=0)


def kernel(x, W1, b1, W2, b2, route):
    x = np.asarray(x)
    route = np.asarray(route)
    tc_tokens = x.shape[0] // N_CORES
    nc = build_moe_pe(tc_tokens, r_tile=512)
    return run_sharded(nc, x, route, tc_tokens,
                       wt=pack_wt(np.asarray(W1), np.asarray(b1),
                                  np.asarray(W2), np.asarray(b2)))

